# revision 41
# baseline (speedup 1.0000x reference)
"""Capsule-routing kernel for Trainium2 (8 NeuronCores, data-parallel over batch).

Math (u_hat never materialized):
  iter1: o1 = 0.1*(sum_n u) @ W_j           -> host (tiny), ships Q2 fp16 hi/lo
  iter t: b = u @ Q_t ; c = softmax_j(b) ; R.T[:,j] = sum_n c[n,j] u[n,:]
          Q_{t+1}[:,j] = K_j @ R.T[:,j]  with K_j = W_j @ W_j.T (host, fp16)
  out = squash(R3 @ W_j)                    -> host epilogue (64x160)

Per core: 8 samples, fp16 u in both layouts (u.T chunks = logits
stationaries, u chunks = R stationaries).  Q rides as fp16 [hi|lo] and each
logits chunk is ONE matmul with a 20-wide mover; the hi+lo add, the per-n
max and the subtraction run per-sample on DVE straight from PSUM, pipelined
behind the next sample's logits.  Each group then runs one batched
exp/z/1z/mul tail.  DMA rides the Sync+GpSimd queues (never ACT/DVE - the
issue instructions would head-of-line block compute) in group-interleaved
order (uT_g, u_g) so each group's routing runs under the next group's
loads; 1-sample tail groups keep the post-DMA critical path short.
"""

import os
import sys

import numpy as np

for _p in ("/opt/trn_rl_repo", "/opt/trn_rl_repo/concourse"):
    if _p not in sys.path and os.path.isdir(_p):
        sys.path.insert(0, _p)

import concourse.bass as bass
import concourse.mybir as mybir
import concourse.tile as tile
from concourse import bacc

F32 = mybir.dt.float32
F32R = mybir.dt.float32r
F16 = mybir.dt.float16
AF = mybir.ActivationFunctionType
AX = mybir.AxisListType
ALU = mybir.AluOpType

N_CORES = 8
B_FULL, N, D = 64, 2048, 128
J, DC = 10, 16
JD = J * DC          # 160
NT = N // 128        # 16 chunks of n per sample
B_LOC = B_FULL // N_CORES  # 8 samples per core
EPS = 1e-7
GROUPS = [(0, 3), (3, 3), (6, 1), (7, 1)]
WARM = 22


def _bcast(ap, extra):
    """Append step-0 (broadcast) dims to an AP."""
    return bass.AP(tensor=ap.tensor, offset=ap.offset,
                   ap=list(ap.ap) + [[0, n] for n in extra])


def build_program():
    nc = bacc.Bacc(None)

    utT_d = nc.declare_dram_parameter("utT", [B_LOC, D, N], F16, isOutput=False)
    unm_d = nc.declare_dram_parameter("unm", [B_LOC, D, NT, D], F16,
                                      isOutput=False)
    q2i_d = nc.declare_dram_parameter("q2i", [D, B_LOC, 2, J], F16,
                                      isOutput=False)
    k_d = nc.declare_dram_parameter("k", [D, J, D], F16, isOutput=False)
    out_d = nc.declare_dram_parameter("out", [D, B_LOC, J], F32, isOutput=True)

    with tile.TileContext(nc) as tc:
        with (
            tc.tile_pool(name="consts", bufs=1) as consts,
            tc.tile_pool(name="big", bufs=1) as big,
            tc.tile_pool(name="sm", bufs=2) as sm,
            tc.tile_pool(name="chain", bufs=2) as chain,
            tc.tile_pool(name="pwarm", bufs=1, space="PSUM") as pwarm,
            tc.tile_pool(name="pbp", bufs=3, space="PSUM") as pbp,
            tc.tile_pool(name="prts", bufs=2, space="PSUM") as prts,
            tc.tile_pool(name="pch", bufs=2, space="PSUM") as pch,
        ):
            k_sb = consts.tile([D, J, D], F16)
            q2i_sb = consts.tile([D, B_LOC, 2, J], F16)
            nc.sync.dma_start(out=q2i_sb[:], in_=q2i_d[:])

            utT = [big.tile([D, NT, D], F16, tag=f"utT{s}", name=f"utT{s}")
                   for s in range(B_LOC)]
            unm = [big.tile([D, NT, D], F16, tag=f"unm{s}", name=f"unm{s}")
                   for s in range(B_LOC)]
            rings = [nc.sync, nc.gpsimd]
            ring_i = 0

            def dma(out, in_):
                nonlocal ring_i
                rings[ring_i % 2].dma_start(out=out, in_=in_)
                ring_i += 1

            # group-interleaved loads: (uT_g then u_g) per group; the two
            # 1-sample tail groups load uT before either unm so only their
            # R-phase hangs off the last arrivals.  q2i rides first (tiny);
            # k slots in behind the first few samples (needed ~20us in).
            load_order = []
            for g0, gsz in GROUPS[:2]:
                load_order += [("t", s) for s in range(g0, g0 + gsz)]
                load_order += [("n", s) for s in range(g0, g0 + gsz)]
            load_order += [("t", 6), ("n", 6), ("t", 7), ("n", 7)]
            for idx, (kind, s) in enumerate(load_order):
                if kind == "t":
                    dma(utT[s][:],
                        utT_d[s, :, :].rearrange("p (t n) -> p t n", t=NT))
                else:
                    dma(unm[s][:], unm_d[s])
                if idx == 3:
                    nc.gpsimd.dma_start(out=k_sb[:], in_=k_d[:])

            # brief PE p-state warmup on the first-arriving const
            warm_ps = pwarm.tile([D, D], F32, tag="warm")
            for _ in range(WARM):
                nc.tensor.matmul(warm_ps[0:120, 0:120], q2i_sb[:, 0:6, :, :],
                                 q2i_sb[:, 0:6, :, :], start=True, stop=True)

            def logits_presm(g0, gsz, q2_of, negm, bs_g, dual):
                """Per-sample logits chunks.  iter2 (dual=True) runs two MMs
                per chunk whose Q-hi/Q-lo products accumulate in PSUM (exact
                Q); iter3 runs one MM with a single fp16 Q.  Either way the
                per-n max and subtraction are one reduce + one add on DVE
                straight from PSUM, pipelined behind later samples."""
                for i in range(gsz):
                    s = g0 + i
                    q2 = q2_of(i)
                    bp = pbp.tile([D, NT, 2, J], F32, tag="bp")
                    for k in range(NT):
                        if dual:
                            nc.tensor.matmul(bp[:, k, 0, :], utT[s][:, k, :],
                                             q2[0], start=True, stop=False)
                            nc.tensor.matmul(bp[:, k, 0, :], utT[s][:, k, :],
                                             q2[1], start=False, stop=True)
                        else:
                            nc.tensor.matmul(bp[:, k, 0, :], utT[s][:, k, :],
                                             q2, start=True, stop=True)
                    nc.vector.reduce_max(negm[:, i, :], bp[:, :, 0, :],
                                         axis=AX.X, negate=True)
                    nc.vector.tensor_add(bs_g[:, i, :, :], bp[:, :, 0, :],
                                         _bcast(negm[:, i, :], [J]))

            def sm_tail(g0, gsz, bs_g):
                e_g = sm.tile([D, gsz, NT, J], F16, tag=f"e{g0}")
                nc.scalar.activation(e_g[:], bs_g[:], AF.Exp)
                z_g = sm.tile([D, gsz, NT], F16, tag=f"z{g0}")
                zr_g = sm.tile([D, gsz, NT], F16, tag=f"r{g0}")
                with nc.allow_low_precision("z in [1,30]; fp16 rel 5e-4 ok"):
                    nc.vector.reduce_sum(z_g[:], e_g[:], axis=AX.X)
                    nc.vector.reciprocal(zr_g[:], z_g[:])
                c_g = sm.tile([D, gsz, NT, J], F16, tag=f"c{g0}")
                nc.vector.tensor_mul(c_g[:], e_g[:], _bcast(zr_g[:], [J]))
                return c_g

            def logits_softmax(g0, gsz, q2_of, dual):
                negm = sm.tile([D, gsz, NT], F32, tag=f"m{g0}")
                bs_g = sm.tile([D, gsz, NT, J], F16, tag=f"s{g0}")
                logits_presm(g0, gsz, q2_of, negm, bs_g, dual)
                return sm_tail(g0, gsz, bs_g)

            def r_group(g0, gsz, c_g, rts_ps):
                for i in range(gsz):
                    s = g0 + i
                    for k in range(NT):
                        nc.tensor.matmul(rts_ps[:, s, :], unm[s][:, k, :],
                                         c_g[:, i, k, :], start=(k == 0),
                                         stop=(k == NT - 1))

            def chain_group(g0, gsz, rts_ps):
                """q2 for iter3: Q[:,j] = K_j @ R.T[:,j], fp16 hi/lo split.
                All PSUM drains ride the otherwise-idle GpSimd engine so the
                ACT queue (exp-only) never head-of-line blocks the chain."""
                rts_sb = chain.tile([D, J, gsz], F16, tag=f"rs{g0}")
                nc.scalar.activation(
                    rts_sb[:],
                    rts_ps[:, g0:g0 + gsz, :].rearrange("p s j -> p j s"),
                    AF.Copy)
                q_ps = pch.tile([D, J, gsz], F32, tag="q_ps")
                for j in range(J):
                    nc.tensor.matmul(q_ps[:, j, :], k_sb[:, j, :],
                                     rts_sb[:, j, :], start=True, stop=True)
                q2_g = chain.tile([D, gsz, J], F16, tag=f"q2{g0}")
                nc.scalar.activation(
                    q2_g[:], q_ps[:].rearrange("p j s -> p s j"), AF.Copy)
                return q2_g

            def drain_out(g0, gsz, rts_ps):
                ob = chain.tile([D, gsz, J], F32, tag=f"ob{g0}")
                nc.scalar.activation(ob[:], rts_ps[:, g0:g0 + gsz, :], AF.Copy)
                nc.gpsimd.dma_start(out=out_d[:, g0:g0 + gsz, :], in_=ob[:])

            rts2 = prts.tile([D, B_LOC, J], F32, tag="rts")
            rts3 = prts.tile([D, B_LOC, J], F32, tag="rts")

            def q2i_of(g0):
                return lambda i: (q2i_sb[:, g0 + i, 0, :],
                                  q2i_sb[:, g0 + i, 1, :])

            def q3_of(q3):
                return lambda i: q3[:, i, :]

            # ready-order emission: each group's softmax tail (DVE/ACT) is
            # covered by the next group's logits or the previous group's
            # iter-3 logits, so the in-order PE queue never sits on a c_g
            # wait without queued work that is already data-ready.
            G = GROUPS
            c2, q3, c3 = {}, {}, {}

            def L2(g):
                g0, gsz = G[g]
                c2[g] = logits_softmax(g0, gsz, q2i_of(g0), dual=True)

            def R2ch(g):
                g0, gsz = G[g]
                r_group(g0, gsz, c2[g], rts2)
                q3[g] = chain_group(g0, gsz, rts2)

            def L3(g):
                g0, gsz = G[g]
                c3[g] = logits_softmax(g0, gsz, q3_of(q3[g]), dual=False)

            def R3(g):
                g0, gsz = G[g]
                r_group(g0, gsz, c3[g], rts3)
                drain_out(g0, gsz, rts3)

            # pin the phase order onto the tile scheduler's virtual clock:
            # rungs 0.1ms apart dwarf the sim's natural ns-scale clock, so
            # the scheduler emits phases strictly in this order (runtime
            # order is still semaphore-driven).  Order = measured DMA
            # arrival order of each phase's gating tensor.
            phases = [
                lambda: L2(0), lambda: R2ch(0), lambda: L2(1),
                lambda: L3(0), lambda: R2ch(1), lambda: L2(2),
                lambda: L3(1), lambda: R2ch(2), lambda: L2(3),
                lambda: R3(0), lambda: L3(2), lambda: R3(1),
                lambda: R2ch(3), lambda: R3(2), lambda: L3(3),
                lambda: R3(3),
            ]
            for pi, emit in enumerate(phases):
                tc.tile_set_cur_wait(0.1 * (pi + 1))
                emit()

    nc.compile()
    return nc


def _f32r(x):
    xi = np.ascontiguousarray(x, np.float32).view(np.uint32).astype(np.int64)
    bias = ((xi >> 12) & 1) + (1 << 11) - 1
    return (((xi + bias) >> 12) << 12).astype(np.uint32).view(np.float32)


def _squash(o):
    s2 = (o ** 2).sum(-1, keepdims=True)
    return o * s2 / ((1.0 + s2) * np.sqrt(s2 + EPS))


_NC = None


def _get_nc():
    global _NC
    if _NC is None:
        _NC = build_program()
    return _NC


def run_sharded(u_vecs: np.ndarray, W: np.ndarray, **kw):
    """Shard over 8 cores, run, return (full_output, BassKernelResults)."""
    from concourse.bass_utils import run_bass_kernel_spmd

    u_vecs = np.ascontiguousarray(u_vecs, dtype=np.float32)
    W = np.ascontiguousarray(W, dtype=np.float32)
    assert u_vecs.shape == (B_FULL, N, D) and W.shape == (D, JD)

    nc = _get_nc()
    Wr = _f32r(W).reshape(D, J, DC)
    # K_j = W_j @ W_j.T, symmetric -> [f, j, g] layout, fp16
    k_arr = np.ascontiguousarray(
        np.einsum('fjd,gjd->fjg', Wr, Wr).astype(np.float16))

    in_maps = []
    for k in range(N_CORES):
        us = u_vecs[k * B_LOC:(k + 1) * B_LOC]          # [8, 2048, 128] f32
        u16 = us.astype(np.float16)
        utT = np.ascontiguousarray(u16.transpose(0, 2, 1))  # [8, 128f, 2048n]
        unm = np.ascontiguousarray(
            u16.reshape(B_LOC, NT, D, D).transpose(0, 2, 1, 3))  # [8,128n,16,128f]
        # host iter-1: o1 = 0.1*(sum_n u) @ W; Q2 = W_j @ o1, as fp16 hi/lo
        o1 = 0.1 * np.einsum('sf,fjd->sjd', us.sum(axis=1), Wr)
        q2 = np.einsum('fjd,sjd->fsj', Wr, o1).astype(np.float32)
        qhi = q2.astype(np.float16)
        qlo = (q2 - qhi.astype(np.float32)).astype(np.float16)
        q2i = np.ascontiguousarray(np.stack([qhi, qlo], axis=2))  # [128,8,2,10]
        in_maps.append({"utT": utT, "unm": unm, "q2i": q2i, "k": k_arr})
    res = run_bass_kernel_spmd(nc, in_maps, core_ids=list(range(N_CORES)), **kw)
    # out: [128 f, 8 s, 10 j] per core = R3.T
    r3 = np.concatenate(
        [res.results[k]["out"].transpose(1, 2, 0) for k in range(N_CORES)],
        axis=0)                                          # [64, 10, 128]
    o3 = np.einsum('sjf,fjd->sjd', r3.astype(np.float64),
                   W.reshape(D, J, DC).astype(np.float64))
    out = _squash(o3.astype(np.float32))
    return out.astype(np.float32), res


def kernel(u_vecs: np.ndarray, W: np.ndarray) -> np.ndarray:
    out, _ = run_sharded(u_vecs, W)
    return out


# revision 42
# speedup vs baseline: 1.0128x; 1.0128x over previous
"""Capsule-routing kernel for Trainium2 (8 NeuronCores, data-parallel over batch).

Math (u_hat never materialized):
  iter1: o1 = 0.1*(sum_n u) @ W_j           -> host (tiny), ships Q2 fp16 hi/lo
  iter t: b = u @ Q_t ; c = softmax_j(b) ; R.T[:,j] = sum_n c[n,j] u[n,:]
          Q_{t+1}[:,j] = K_j @ R.T[:,j]  with K_j = W_j @ W_j.T (host, fp16)
  out = squash(R3 @ W_j)                    -> host epilogue (64x160)

Per core: 8 samples, fp16 u in both layouts (u.T chunks = logits
stationaries, u chunks = R stationaries).  Q rides as fp16 [hi|lo] and each
logits chunk is ONE matmul with a 20-wide mover; the hi+lo add, the per-n
max and the subtraction run per-sample on DVE straight from PSUM, pipelined
behind the next sample's logits.  Each group then runs one batched
exp/z/1z/mul tail.  DMA rides the Sync+GpSimd queues (never ACT/DVE - the
issue instructions would head-of-line block compute) in group-interleaved
order (uT_g, u_g) so each group's routing runs under the next group's
loads; 1-sample tail groups keep the post-DMA critical path short.
"""

import os
import sys

import numpy as np

for _p in ("/opt/trn_rl_repo", "/opt/trn_rl_repo/concourse"):
    if _p not in sys.path and os.path.isdir(_p):
        sys.path.insert(0, _p)

import concourse.bass as bass
import concourse.mybir as mybir
import concourse.tile as tile
from concourse import bacc

F32 = mybir.dt.float32
F32R = mybir.dt.float32r
F16 = mybir.dt.float16
AF = mybir.ActivationFunctionType
AX = mybir.AxisListType
ALU = mybir.AluOpType

N_CORES = 8
B_FULL, N, D = 64, 2048, 128
J, DC = 10, 16
JD = J * DC          # 160
NT = N // 128        # 16 chunks of n per sample
B_LOC = B_FULL // N_CORES  # 8 samples per core
EPS = 1e-7
GROUPS = [(0, 3), (3, 3), (6, 1), (7, 1)]
WARM = 4


def _bcast(ap, extra):
    """Append step-0 (broadcast) dims to an AP."""
    return bass.AP(tensor=ap.tensor, offset=ap.offset,
                   ap=list(ap.ap) + [[0, n] for n in extra])


def build_program():
    nc = bacc.Bacc(None)

    utT_d = nc.declare_dram_parameter("utT", [B_LOC, D, N], F16, isOutput=False)
    unm_d = nc.declare_dram_parameter("unm", [B_LOC, D, NT, D], F16,
                                      isOutput=False)
    q2i_d = nc.declare_dram_parameter("q2i", [D, B_LOC, 2, J], F16,
                                      isOutput=False)
    k_d = nc.declare_dram_parameter("k", [D, J, D], F16, isOutput=False)
    out_d = nc.declare_dram_parameter("out", [D, B_LOC, J], F32, isOutput=True)

    with tile.TileContext(nc) as tc:
        with (
            tc.tile_pool(name="consts", bufs=1) as consts,
            tc.tile_pool(name="big", bufs=1) as big,
            tc.tile_pool(name="sm", bufs=2) as sm,
            tc.tile_pool(name="chain", bufs=2) as chain,
            tc.tile_pool(name="pwarm", bufs=1, space="PSUM") as pwarm,
            tc.tile_pool(name="pbp", bufs=3, space="PSUM") as pbp,
            tc.tile_pool(name="prts", bufs=2, space="PSUM") as prts,
            tc.tile_pool(name="pch", bufs=2, space="PSUM") as pch,
        ):
            k_sb = consts.tile([D, J, D], F16)
            q2i_sb = consts.tile([D, B_LOC, 2, J], F16)
            nc.sync.dma_start(out=q2i_sb[:], in_=q2i_d[:])

            utT = [big.tile([D, NT, D], F16, tag=f"utT{s}", name=f"utT{s}")
                   for s in range(B_LOC)]
            unm = [big.tile([D, NT, D], F16, tag=f"unm{s}", name=f"unm{s}")
                   for s in range(B_LOC)]
            rings = [nc.sync, nc.gpsimd]
            ring_i = 0

            def dma(out, in_):
                nonlocal ring_i
                rings[ring_i % 2].dma_start(out=out, in_=in_)
                ring_i += 1

            # group-interleaved loads: (uT_g then u_g) per group; the two
            # 1-sample tail groups load uT before either unm so only their
            # R-phase hangs off the last arrivals.  q2i rides first (tiny);
            # k slots in behind the first few samples (needed ~20us in).
            load_order = []
            for g0, gsz in GROUPS[:2]:
                load_order += [("t", s) for s in range(g0, g0 + gsz)]
                load_order += [("n", s) for s in range(g0, g0 + gsz)]
            load_order += [("t", 6), ("n", 6), ("t", 7), ("n", 7)]
            for idx, (kind, s) in enumerate(load_order):
                if kind == "t":
                    dma(utT[s][:],
                        utT_d[s, :, :].rearrange("p (t n) -> p t n", t=NT))
                else:
                    dma(unm[s][:], unm_d[s])
                if idx == 3:
                    nc.gpsimd.dma_start(out=k_sb[:], in_=k_d[:])

            # brief PE p-state warmup on the first-arriving const
            warm_ps = pwarm.tile([D, D], F32, tag="warm")
            for _ in range(WARM):
                nc.tensor.matmul(warm_ps[0:120, 0:120], q2i_sb[:, 0:6, :, :],
                                 q2i_sb[:, 0:6, :, :], start=True, stop=True)

            def logits_presm(g0, gsz, q2_of, negm, bs_g, dual):
                """Per-sample logits chunks.  iter2 (dual=True) runs two MMs
                per chunk whose Q-hi/Q-lo products accumulate in PSUM (exact
                Q); iter3 runs one MM with a single fp16 Q.  Either way the
                per-n max and subtraction are one reduce + one add on DVE
                straight from PSUM, pipelined behind later samples."""
                for i in range(gsz):
                    s = g0 + i
                    q2 = q2_of(i)
                    bp = pbp.tile([D, NT, 2, J], F32, tag="bp")
                    for k in range(NT):
                        if dual:
                            nc.tensor.matmul(bp[:, k, 0, :], utT[s][:, k, :],
                                             q2[0], start=True, stop=False)
                            nc.tensor.matmul(bp[:, k, 0, :], utT[s][:, k, :],
                                             q2[1], start=False, stop=True)
                        else:
                            nc.tensor.matmul(bp[:, k, 0, :], utT[s][:, k, :],
                                             q2, start=True, stop=True)
                    nc.vector.reduce_max(negm[:, i, :], bp[:, :, 0, :],
                                         axis=AX.X, negate=True)
                    nc.vector.tensor_add(bs_g[:, i, :, :], bp[:, :, 0, :],
                                         _bcast(negm[:, i, :], [J]))

            def sm_tail(g0, gsz, bs_g):
                e_g = sm.tile([D, gsz, NT, J], F16, tag=f"e{g0}")
                nc.scalar.activation(e_g[:], bs_g[:], AF.Exp)
                z_g = sm.tile([D, gsz, NT], F16, tag=f"z{g0}")
                zr_g = sm.tile([D, gsz, NT], F16, tag=f"r{g0}")
                with nc.allow_low_precision("z in [1,30]; fp16 rel 5e-4 ok"):
                    nc.vector.reduce_sum(z_g[:], e_g[:], axis=AX.X)
                    nc.vector.reciprocal(zr_g[:], z_g[:])
                c_g = sm.tile([D, gsz, NT, J], F16, tag=f"c{g0}")
                nc.vector.tensor_mul(c_g[:], e_g[:], _bcast(zr_g[:], [J]))
                return c_g

            def logits_softmax(g0, gsz, q2_of, dual):
                negm = sm.tile([D, gsz, NT], F32, tag=f"m{g0}")
                bs_g = sm.tile([D, gsz, NT, J], F16, tag=f"s{g0}")
                logits_presm(g0, gsz, q2_of, negm, bs_g, dual)
                return sm_tail(g0, gsz, bs_g)

            def r_group(g0, gsz, c_g, rts_ps):
                for i in range(gsz):
                    s = g0 + i
                    for k in range(NT):
                        nc.tensor.matmul(rts_ps[:, s, :], unm[s][:, k, :],
                                         c_g[:, i, k, :], start=(k == 0),
                                         stop=(k == NT - 1))

            def chain_group(g0, gsz, rts_ps):
                """q2 for iter3: Q[:,j] = K_j @ R.T[:,j], fp16 hi/lo split.
                All PSUM drains ride the otherwise-idle GpSimd engine so the
                ACT queue (exp-only) never head-of-line blocks the chain."""
                rts_sb = chain.tile([D, J, gsz], F16, tag=f"rs{g0}")
                nc.scalar.activation(
                    rts_sb[:],
                    rts_ps[:, g0:g0 + gsz, :].rearrange("p s j -> p j s"),
                    AF.Copy)
                q_ps = pch.tile([D, J, gsz], F32, tag="q_ps")
                for j in range(J):
                    nc.tensor.matmul(q_ps[:, j, :], k_sb[:, j, :],
                                     rts_sb[:, j, :], start=True, stop=True)
                q2_g = chain.tile([D, gsz, J], F16, tag=f"q2{g0}")
                nc.scalar.activation(
                    q2_g[:], q_ps[:].rearrange("p j s -> p s j"), AF.Copy)
                return q2_g

            def drain_out(g0, gsz, rts_ps):
                ob = chain.tile([D, gsz, J], F32, tag=f"ob{g0}")
                nc.scalar.activation(ob[:], rts_ps[:, g0:g0 + gsz, :], AF.Copy)
                nc.gpsimd.dma_start(out=out_d[:, g0:g0 + gsz, :], in_=ob[:])

            rts2 = prts.tile([D, B_LOC, J], F32, tag="rts")
            rts3 = prts.tile([D, B_LOC, J], F32, tag="rts")

            def q2i_of(g0):
                return lambda i: (q2i_sb[:, g0 + i, 0, :],
                                  q2i_sb[:, g0 + i, 1, :])

            def q3_of(q3):
                return lambda i: q3[:, i, :]

            # ready-order emission: each group's softmax tail (DVE/ACT) is
            # covered by the next group's logits or the previous group's
            # iter-3 logits, so the in-order PE queue never sits on a c_g
            # wait without queued work that is already data-ready.
            G = GROUPS
            c2, q3, c3 = {}, {}, {}

            def L2(g):
                g0, gsz = G[g]
                c2[g] = logits_softmax(g0, gsz, q2i_of(g0), dual=True)

            def R2ch(g):
                g0, gsz = G[g]
                r_group(g0, gsz, c2[g], rts2)
                q3[g] = chain_group(g0, gsz, rts2)

            def L3(g):
                g0, gsz = G[g]
                c3[g] = logits_softmax(g0, gsz, q3_of(q3[g]), dual=False)

            def R3(g):
                g0, gsz = G[g]
                r_group(g0, gsz, c3[g], rts3)
                drain_out(g0, gsz, rts3)

            # pin the phase order onto the tile scheduler's virtual clock:
            # rungs 0.1ms apart dwarf the sim's natural ns-scale clock, so
            # the scheduler emits phases strictly in this order (runtime
            # order is still semaphore-driven).  Order = measured DMA
            # arrival order of each phase's gating tensor.
            phases = [
                lambda: L2(0), lambda: R2ch(0), lambda: L2(1),
                lambda: L3(0), lambda: R2ch(1), lambda: L2(2),
                lambda: L3(1), lambda: R2ch(2), lambda: L2(3),
                lambda: R3(0), lambda: L3(2), lambda: R3(1),
                lambda: R2ch(3), lambda: R3(2), lambda: L3(3),
                lambda: R3(3),
            ]
            for pi, emit in enumerate(phases):
                tc.tile_set_cur_wait(0.1 * (pi + 1))
                emit()

    nc.compile()
    return nc


def _f32r(x):
    xi = np.ascontiguousarray(x, np.float32).view(np.uint32).astype(np.int64)
    bias = ((xi >> 12) & 1) + (1 << 11) - 1
    return (((xi + bias) >> 12) << 12).astype(np.uint32).view(np.float32)


def _squash(o):
    s2 = (o ** 2).sum(-1, keepdims=True)
    return o * s2 / ((1.0 + s2) * np.sqrt(s2 + EPS))


_NC = None


def _get_nc():
    global _NC
    if _NC is None:
        _NC = build_program()
    return _NC


def run_sharded(u_vecs: np.ndarray, W: np.ndarray, **kw):
    """Shard over 8 cores, run, return (full_output, BassKernelResults)."""
    from concourse.bass_utils import run_bass_kernel_spmd

    u_vecs = np.ascontiguousarray(u_vecs, dtype=np.float32)
    W = np.ascontiguousarray(W, dtype=np.float32)
    assert u_vecs.shape == (B_FULL, N, D) and W.shape == (D, JD)

    nc = _get_nc()
    Wr = _f32r(W).reshape(D, J, DC)
    # K_j = W_j @ W_j.T, symmetric -> [f, j, g] layout, fp16
    k_arr = np.ascontiguousarray(
        np.einsum('fjd,gjd->fjg', Wr, Wr).astype(np.float16))

    in_maps = []
    for k in range(N_CORES):
        us = u_vecs[k * B_LOC:(k + 1) * B_LOC]          # [8, 2048, 128] f32
        u16 = us.astype(np.float16)
        utT = np.ascontiguousarray(u16.transpose(0, 2, 1))  # [8, 128f, 2048n]
        unm = np.ascontiguousarray(
            u16.reshape(B_LOC, NT, D, D).transpose(0, 2, 1, 3))  # [8,128n,16,128f]
        # host iter-1: o1 = 0.1*(sum_n u) @ W; Q2 = W_j @ o1, as fp16 hi/lo
        o1 = 0.1 * np.einsum('sf,fjd->sjd', us.sum(axis=1), Wr)
        q2 = np.einsum('fjd,sjd->fsj', Wr, o1).astype(np.float32)
        qhi = q2.astype(np.float16)
        qlo = (q2 - qhi.astype(np.float32)).astype(np.float16)
        q2i = np.ascontiguousarray(np.stack([qhi, qlo], axis=2))  # [128,8,2,10]
        in_maps.append({"utT": utT, "unm": unm, "q2i": q2i, "k": k_arr})
    res = run_bass_kernel_spmd(nc, in_maps, core_ids=list(range(N_CORES)), **kw)
    # out: [128 f, 8 s, 10 j] per core = R3.T
    r3 = np.concatenate(
        [res.results[k]["out"].transpose(1, 2, 0) for k in range(N_CORES)],
        axis=0)                                          # [64, 10, 128]
    o3 = np.einsum('sjf,fjd->sjd', r3.astype(np.float64),
                   W.reshape(D, J, DC).astype(np.float64))
    out = _squash(o3.astype(np.float32))
    return out.astype(np.float32), res


def kernel(u_vecs: np.ndarray, W: np.ndarray) -> np.ndarray:
    out, _ = run_sharded(u_vecs, W)
    return out
